# revision 10
# baseline (speedup 1.0000x reference)
"""Trainium2 Bass kernel for a dense pre-norm transformer block.

Reference computation (per batch element, all f32 inputs):
    xn   = LN(x; ln1_g, ln1_b)
    qkv  = xn @ w_qkv.T                   # [N, 3C]
    attn = softmax(q k^T / sqrt(D))       # [H, N, N]  (also an output)
    y    = (attn @ v) @ w_proj.T + b_proj
    x1   = x + y
    h    = gelu(LN(x1) @ w_fc1.T + b_fc1)
    out  = x1 + h @ w_fc2.T + b_fc2
Returns (out, attn).

Sharding: pure data parallel — batch 8 over the 8 NeuronCores, one batch
element per core, full weights replicated. No collectives.

Single-core dataflow (per core, N=1024 tokens, C=1024, H=16 heads, D=64):
  - LN stats in token-major, then PE-transpose the normalized activations
    into feature-major xnT [C, N] (bf16) for matmuls.
  - qT,kT computed feature-major; v computed token-major [N, C].
  - scores are computed TWICE (q-major for softmax/attn-output, k-major
    for attn@v) to avoid transposing the 16M-element attention matrix.
    exp() skips max-subtraction: logits ~ N(0,1), safe in f32/bf16.
  - attn@v consumes unnormalized exp^T; the 1/rowsum normalization is
    applied to y^T feature-major via r_exp (rowsum reciprocals broadcast
    across each head's 64 channels with a tiny selection matmul).
  - proj/fc1/fc2 are standard K-accumulated matmuls; residuals in f32.
Weights are pre-transposed and cast to bf16 on the host.
"""

import sys

if "/opt/trn_rl_repo" not in sys.path:
    sys.path.insert(0, "/opt/trn_rl_repo")

from contextlib import ExitStack

import ml_dtypes
import numpy as np

import concourse.bacc as bacc
import concourse.mybir as mybir
import concourse.tile as tile
from concourse import masks
from concourse.bass_utils import run_bass_kernel_spmd

AF = mybir.ActivationFunctionType
ALU = mybir.AluOpType
AX = mybir.AxisListType
F32 = mybir.dt.float32
BF16 = mybir.dt.bfloat16

P = 128
N = 1024  # tokens per core
C = 1024  # model dim
H = 16  # heads
D = 64  # head dim
HID = 4096
NT = N // P  # 8 token tiles
CT = C // P  # 8 channel tiles
HT = HID // P  # 32 hidden tiles
B = 8  # batch == n_cores
EPS = 1e-5
SCALE = float(D) ** -0.5

# CoreSim does not implement the Gelu LUT; sim_check flips this to validate
# everything else with a sigmoid-approx gelu (HW always uses AF.Gelu).
SIM_COMPAT = False
# Bisection aid: subset of stages to emit, e.g. "123" = LN1+QKV only.
STAGES = "12345678"


def build_graph():
    nc = bacc.Bacc("TRN2", target_bir_lowering=False, debug=False, num_devices=B)

    x_d = nc.declare_dram_parameter("x", [N, C], F32, isOutput=False)
    wqkvT_d = nc.declare_dram_parameter("wqkvT", [C, 3 * C], BF16, isOutput=False)
    wprojT_d = nc.declare_dram_parameter("wprojT", [C, C], BF16, isOutput=False)
    wfc1T_d = nc.declare_dram_parameter("wfc1T", [C, HID], BF16, isOutput=False)
    wfc2T_d = nc.declare_dram_parameter("wfc2T", [HID, C], BF16, isOutput=False)
    ln1g_d = nc.declare_dram_parameter("ln1g", [P, CT], F32, isOutput=False)
    ln1b_d = nc.declare_dram_parameter("ln1b", [P, CT], F32, isOutput=False)
    ln2g_d = nc.declare_dram_parameter("ln2g", [P, CT], F32, isOutput=False)
    ln2b_d = nc.declare_dram_parameter("ln2b", [P, CT], F32, isOutput=False)
    bfc1_d = nc.declare_dram_parameter("bfc1", [P, HT], F32, isOutput=False)
    bprojB_d = nc.declare_dram_parameter("bprojB", [P, C], F32, isOutput=False)
    bfc2B_d = nc.declare_dram_parameter("bfc2B", [P, C], F32, isOutput=False)
    sel2_d = nc.declare_dram_parameter("sel2", [2, P], F32, isOutput=False)
    out_d = nc.declare_dram_parameter("out", [N, C], F32, isOutput=True)
    attn_d = nc.declare_dram_parameter("attn", [H, N, N], F32, isOutput=True)

    with tile.TileContext(nc, pool_alloc_mode="queue") as tc, ExitStack() as ctx:
        # ---------------- constant / persistent pools ----------------
        const = ctx.enter_context(tc.tile_pool(name="const", bufs=1))
        ident_bf = const.tile([P, P], BF16, tag="ident_bf", name="ident_bf")
        ident_f32 = const.tile([P, P], F32, tag="ident_f32", name="ident_f32")
        masks.make_identity(nc, ident_bf[:])
        masks.make_identity(nc, ident_f32[:])
        ln1g = const.tile([P, CT], F32, tag="ln1g", name="ln1g")
        ln1b = const.tile([P, CT], F32, tag="ln1b", name="ln1b")
        ln2g = const.tile([P, CT], F32, tag="ln2g", name="ln2g")
        ln2b = const.tile([P, CT], F32, tag="ln2b", name="ln2b")
        bfc1 = const.tile([P, HT], F32, tag="bfc1", name="bfc1")
        bprojB = const.tile([P, C], F32, tag="bprojB", name="bprojB")
        bfc2B = const.tile([P, C], F32, tag="bfc2B", name="bfc2B")
        sel2 = const.tile([2, P], F32, tag="sel2", name="sel2")
        eps_t = const.tile([P, 1], F32, tag="eps", name="eps")
        nc.gpsimd.memset(eps_t[:], EPS)
        nc.sync.dma_start(ln1g[:], ln1g_d[:])
        nc.sync.dma_start(ln1b[:], ln1b_d[:])
        nc.sync.dma_start(ln2g[:], ln2g_d[:])
        nc.sync.dma_start(ln2b[:], ln2b_d[:])
        nc.sync.dma_start(bfc1[:], bfc1_d[:])
        nc.sync.dma_start(bprojB[:], bprojB_d[:])
        nc.sync.dma_start(bfc2B[:], bfc2B_d[:])
        nc.sync.dma_start(sel2[:], sel2_d[:])

        # x sheet: token-major residual stream, f32. Updated in place at
        # the attention residual, so it holds x then x1.
        xpool = ctx.enter_context(tc.tile_pool(name="x", bufs=1))
        x_sb = [xpool.tile([P, C], F32, tag=f"x{t}", name=f"x{t}") for t in range(NT)]
        for t in range(NT):
            nc.sync.dma_start(x_sb[t][:], x_d[t * P : (t + 1) * P, :])

        # ---------------- LN + transpose (shared helper) ----------------
        def layernorm_transpose(src_sheet, g, b, dst_sheet, ln_pool, psum_tr):
            """Token-major LN of src_sheet -> feature-major bf16 dst_sheet."""
            for t in range(NT):
                mu = ln_pool.tile([P, 1], F32, tag="mu", name="mu")
                nc.vector.reduce_sum(out=mu[:], in_=src_sheet[t][:], axis=AX.X)
                nc.vector.tensor_scalar_mul(mu[:], mu[:], 1.0 / C)
                z = ln_pool.tile([P, C], F32, tag="z", name="z")
                nc.vector.tensor_scalar_sub(z[:], src_sheet[t][:], mu[:])
                sq = ln_pool.tile([P, C], F32, tag="sq", name="sq")
                var = ln_pool.tile([P, 1], F32, tag="var", name="var")
                # (tensor_tensor_reduce is broken on this runtime; use the
                # ScalarE Square LUT with row-sum accumulation instead)
                nc.scalar.activation(sq[:], z[:], AF.Square, accum_out=var[:])
                # rstd = 1/sqrt(var/C + eps)
                nc.scalar.activation(var[:], var[:], AF.Sqrt, bias=eps_t[:], scale=1.0 / C)
                nc.vector.reciprocal(var[:], var[:])
                zb = ln_pool.tile([P, C], BF16, tag="zb", name="zb")
                nc.vector.tensor_scalar_mul(zb[:], z[:], var[:])
                for c in range(CT):
                    pt = psum_tr.tile([P, P], BF16, tag="tr", name="tr")
                    nc.tensor.transpose(pt[:], zb[:, c * P : (c + 1) * P], ident_bf[:])
                    nc.vector.tensor_scalar(
                        out=dst_sheet[c][:, t * P : (t + 1) * P],
                        in0=pt[:],
                        scalar1=g[:, c : c + 1],
                        scalar2=b[:, c : c + 1],
                        op0=ALU.mult,
                        op1=ALU.add,
                    )

        # qT/kT/v live from stage 3 into stage 4
        es_qkv = ExitStack()
        qkT_pool = es_qkv.enter_context(tc.tile_pool(name="qkT", bufs=1))
        es_y = ExitStack()
        qT = [qkT_pool.tile([P, N], BF16, tag=f"qT{j}", name=f"qT{j}") for j in range(CT)]
        kT = [qkT_pool.tile([P, N], BF16, tag=f"kT{j}", name=f"kT{j}") for j in range(CT)]
        v_sb = [qkT_pool.tile([P, C], BF16, tag=f"v{t}", name=f"v{t}") for t in range(NT)]

        # ===== stages 1-3: LN1 -> xnT -> QKV =====
        with tc.tile_pool(name="xnT", bufs=1) as xnT_pool:
            xnT = [xnT_pool.tile([P, N], BF16, tag=f"xnT{c}", name=f"xnT{c}") for c in range(CT)]
            with tc.tile_pool(name="ln1", bufs=2) as ln_pool, tc.tile_pool(
                name="ps_tr1", bufs=4, space="PSUM"
            ) as psum_tr:
                if "1" in STAGES:
                    layernorm_transpose(x_sb, ln1g, ln1b, xnT, ln_pool, psum_tr)

            with tc.tile_pool(name="wqkv", bufs=1) as wq_pool, tc.tile_pool(
                name="ps_qkv", bufs=2, space="PSUM"
            ) as ps_qkv:
                wqkvT = [
                    wq_pool.tile([P, 3 * C], BF16, tag=f"wqkvT{c}", name=f"wqkvT{c}") for c in range(CT)
                ]
                if "3" not in STAGES:
                    wqkvT = wqkvT[:0]
                for c in range(len(wqkvT) and CT):
                    nc.sync.dma_start(wqkvT[c][:], wqkvT_d[c * P : (c + 1) * P, :])
                # q^T and k^T, feature-major: out[f, t] over 16 f-tiles
                for f in range((2 * CT) if "3" in STAGES else 0):
                    dst = qT[f] if f < CT else kT[f - CT]
                    ps = ps_qkv.tile([P, N], F32, tag="ps", name="ps")
                    for c in range(CT):
                        for hh in range(2):
                            nc.tensor.matmul(
                                ps[:, hh * 512 : (hh + 1) * 512],
                                lhsT=wqkvT[c][:, f * P : (f + 1) * P],
                                rhs=xnT[c][:, hh * 512 : (hh + 1) * 512],
                                start=(c == 0),
                                stop=(c == CT - 1),
                            )
                    nc.vector.tensor_copy(dst[:], ps[:])
                # v, token-major: out[t, vf]
                for t in range(NT if "3" in STAGES else 0):
                    ps = ps_qkv.tile([P, N], F32, tag="ps", name="ps")
                    for c in range(CT):
                        for hh in range(2):
                            nc.tensor.matmul(
                                ps[:, hh * 512 : (hh + 1) * 512],
                                lhsT=xnT[c][:, t * P : (t + 1) * P],
                                rhs=wqkvT[c][
                                    :, 2 * C + hh * 512 : 2 * C + (hh + 1) * 512
                                ],
                                start=(c == 0),
                                stop=(c == CT - 1),
                            )
                    nc.vector.tensor_copy(v_sb[t][:], ps[:])

        # ===== stages 4-5: attention + proj + residual =====
        yT_pool = es_y.enter_context(tc.tile_pool(name="yT", bufs=1))
        yT = [yT_pool.tile([P, N], BF16, tag=f"yT{j}", name=f"yT{j}") for j in range(CT)]
        wproj_pool = es_y.enter_context(tc.tile_pool(name="wproj", bufs=1))
        wprojT = [wproj_pool.tile([P, C], BF16, tag=f"wprojT{c}", name=f"wprojT{c}") for c in range(CT)]
        for c in range(CT):
            nc.sync.dma_start(wprojT[c][:], wprojT_d[c * P : (c + 1) * P, :])

        with tc.tile_pool(name="at_sb", bufs=1) as at_sb, tc.tile_pool(
            name="at_r", bufs=2
        ) as at_r, tc.tile_pool(name="expq", bufs=4) as expq_pool, tc.tile_pool(
            name="expT", bufs=4
        ) as expT_pool, tc.tile_pool(name="at_out", bufs=4) as at_out, tc.tile_pool(
            name="ps_s", bufs=2, space="PSUM"
        ) as ps_s, tc.tile_pool(
            name="ps_sT", bufs=2, space="PSUM"
        ) as ps_sT, tc.tile_pool(
            name="ps_y", bufs=2, space="PSUM"
        ) as ps_y:
            sums0 = at_sb.tile([P, NT * H], F32, tag="sums0", name="sums0")  # col qt*16+h
            sums1 = at_sb.tile([P, NT * H], F32, tag="sums1", name="sums1")
            r_all = at_sb.tile([P, NT * H], F32, tag="r_all", name="r_all")
            for j in range(H // 2 if "4" in STAGES else 0):  # head pair
                # --- q-major scores: softmax row sums + attn output ---
                for a in range(2):
                    h = 2 * j + a
                    r0 = a * D
                    for qt in range(NT):
                        col = qt * H + h
                        e = expq_pool.tile([P, N], BF16, tag="e", name="e")
                        for kh in range(2):
                            ps = ps_s.tile([P, 512], F32, tag="s", name="s")
                            nc.tensor.matmul(
                                ps[:],
                                lhsT=qT[j][r0 : r0 + D, qt * P : (qt + 1) * P],
                                rhs=kT[j][r0 : r0 + D, kh * 512 : (kh + 1) * 512],
                                start=True,
                                stop=True,
                            )
                            sums = sums0 if kh == 0 else sums1
                            nc.scalar.activation(
                                e[:, kh * 512 : (kh + 1) * 512],
                                ps[:],
                                AF.Exp,
                                scale=SCALE,
                                accum_out=sums[:, col : col + 1],
                            )
                        nc.vector.tensor_add(
                            r_all[:, col : col + 1],
                            sums0[:, col : col + 1],
                            sums1[:, col : col + 1],
                        )
                        nc.vector.reciprocal(
                            r_all[:, col : col + 1], r_all[:, col : col + 1]
                        )
                        at = at_out.tile([P, N], F32, tag="at", name="at")
                        nc.vector.tensor_scalar_mul(
                            at[:], e[:], r_all[:, col : col + 1]
                        )
                        nc.sync.dma_start(attn_d[h, qt * P : (qt + 1) * P, :], at[:])

                # --- k-major scores -> exp^T -> attn @ v (accumulate) ---
                # Heads run SEQUENTIALLY per psum bank: start=True clears the
                # has_written flags of the whole bank, so the two heads'
                # accumulation groups must not interleave. Head a=1 starts
                # after head a=0 stopped; a=0's finished values are untouched.
                py = [ps_y.tile([P, 512], F32, tag="y", name="y") for _ in range(2)]
                for a in range(2):
                    h = 2 * j + a
                    r0 = a * D
                    for kt in range(NT):
                        eT = expT_pool.tile([P, N], BF16, tag="eT", name="eT")
                        for qh in range(2):
                            ps = ps_sT.tile([P, 512], F32, tag="sT", name="sT")
                            nc.tensor.matmul(
                                ps[:],
                                lhsT=kT[j][r0 : r0 + D, kt * P : (kt + 1) * P],
                                rhs=qT[j][r0 : r0 + D, qh * 512 : (qh + 1) * 512],
                                start=True,
                                stop=True,
                            )
                            nc.scalar.activation(
                                eT[:, qh * 512 : (qh + 1) * 512], ps[:], AF.Exp,
                                scale=SCALE,
                            )
                        for qh in range(2):
                            nc.tensor.matmul(
                                py[qh][a * D : (a + 1) * D, :],
                                lhsT=v_sb[kt][:, h * D : (h + 1) * D],
                                rhs=eT[:, qh * 512 : (qh + 1) * 512],
                                start=(kt == 0),
                                stop=(kt == NT - 1),
                            )

                # --- r_exp: broadcast 1/rowsum over each head's channels ---
                rT = at_r.tile([2, N], F32, tag="rT", name="rT")
                for qt in range(NT):
                    c0 = qt * H + 2 * j
                    pst = ps_s.tile([P, 512], F32, tag="s", name="s")
                    nc.tensor.transpose(
                        pst[0:2, 0:P], r_all[:, c0 : c0 + 2], ident_f32[:]
                    )
                    nc.vector.tensor_copy(rT[:, qt * P : (qt + 1) * P], pst[0:2, 0:P])
                rexp = at_r.tile([P, N], F32, tag="rexp", name="rexp")
                for qh in range(2):
                    psr = ps_s.tile([P, 512], F32, tag="s", name="s")
                    nc.tensor.matmul(
                        psr[:],
                        lhsT=sel2[:],
                        rhs=rT[:, qh * 512 : (qh + 1) * 512],
                        start=True,
                        stop=True,
                    )
                    nc.vector.tensor_copy(rexp[:, qh * 512 : (qh + 1) * 512], psr[:])
                # --- yT = normalized attention output, feature-major ---
                for qh in range(2):
                    nc.vector.tensor_mul(
                        yT[j][:, qh * 512 : (qh + 1) * 512],
                        py[qh][:],
                        rexp[:, qh * 512 : (qh + 1) * 512],
                    )

        # ----- stage 5: proj + residual (in place on x sheet) -----
        with tc.tile_pool(name="ps_proj", bufs=2, space="PSUM") as ps_proj:
            for t in range(NT if "5" in STAGES else 0):
                pp = ps_proj.tile([P, C], F32, tag="pp", name="pp")
                for c in range(CT):
                    for ch in range(2):
                        nc.tensor.matmul(
                            pp[:, ch * 512 : (ch + 1) * 512],
                            lhsT=yT[c][:, t * P : (t + 1) * P],
                            rhs=wprojT[c][:, ch * 512 : (ch + 1) * 512],
                            start=(c == 0),
                            stop=(c == CT - 1),
                        )
                nc.vector.tensor_add(x_sb[t][:], pp[:], x_sb[t][:])
                nc.vector.tensor_add(x_sb[t][:], x_sb[t][:], bprojB[:])

        es_y.close()  # free yT / wproj
        es_qkv.close()  # free qT/kT/v

        # ===== stages 6-8: LN2 -> fc1+gelu -> fc2 + residual =====
        es_mlp = ExitStack()
        h_pool = es_mlp.enter_context(tc.tile_pool(name="h", bufs=1))
        h_sb = [h_pool.tile([P, N], BF16, tag=f"h{i}", name=f"h{i}") for i in range(HT)]

        with tc.tile_pool(name="xn2T", bufs=1) as xn2T_pool:
            xn2T = [xn2T_pool.tile([P, N], BF16, tag=f"xn2T{c}", name=f"xn2T{c}") for c in range(CT)]
            with tc.tile_pool(name="ln2", bufs=2) as ln_pool, tc.tile_pool(
                name="ps_tr2", bufs=4, space="PSUM"
            ) as psum_tr:
                if "6" in STAGES:
                    layernorm_transpose(x_sb, ln2g, ln2b, xn2T, ln_pool, psum_tr)

            with tc.tile_pool(name="wfc1", bufs=2) as wfc1_pool, tc.tile_pool(
                name="ps_fc1", bufs=2, space="PSUM"
            ) as ps_fc1, tc.tile_pool(name="ln2_gelu", bufs=2) as ln2_gelu_pool:
                for g in range(8 if "7" in STAGES else 0):  # groups of 4 hidden tiles
                    wg = [
                        wfc1_pool.tile([P, 512], BF16, tag=f"w{c}", name=f"w{c}") for c in range(CT)
                    ]
                    for c in range(CT):
                        nc.sync.dma_start(
                            wg[c][:],
                            wfc1T_d[c * P : (c + 1) * P, g * 512 : (g + 1) * 512],
                        )
                    for hi in range(4):
                        hf = g * 4 + hi
                        ps = ps_fc1.tile([P, N], F32, tag="ps", name="ps")
                        for c in range(CT):
                            for th in range(2):
                                nc.tensor.matmul(
                                    ps[:, th * 512 : (th + 1) * 512],
                                    lhsT=wg[c][:, hi * P : (hi + 1) * P],
                                    rhs=xn2T[c][:, th * 512 : (th + 1) * 512],
                                    start=(c == 0),
                                    stop=(c == CT - 1),
                                )
                        if SIM_COMPAT:
                            u = ln2_gelu_pool.tile([P, N], F32, tag="u", name="u")
                            nc.vector.tensor_scalar_add(
                                u[:], ps[:], bfc1[:, hf : hf + 1]
                            )
                            s_t = ln2_gelu_pool.tile([P, N], F32, tag="s_t", name="s_t")
                            nc.scalar.activation(s_t[:], u[:], AF.Sigmoid, scale=1.702)
                            nc.vector.tensor_mul(h_sb[hf][:], u[:], s_t[:])
                        else:
                            nc.scalar.activation(
                                h_sb[hf][:],
                                ps[:],
                                AF.Gelu,
                                bias=bfc1[:, hf : hf + 1],
                                scale=1.0,
                            )

        with tc.tile_pool(name="wfc2", bufs=1) as wfc2_pool, tc.tile_pool(
            name="ps_fc2", bufs=2, space="PSUM"
        ) as ps_fc2, tc.tile_pool(name="out_sb", bufs=4) as out_pool:
            wfc2T = [wfc2_pool.tile([P, C], BF16, tag=f"wfc2T{i}", name=f"wfc2T{i}") for i in range(HT)]
            for i in range(HT if "8" in STAGES else 0):
                nc.sync.dma_start(wfc2T[i][:], wfc2T_d[i * P : (i + 1) * P, :])
            for t in range(NT if "8" in STAGES else 0):
                ps = ps_fc2.tile([P, C], F32, tag="ps", name="ps")
                for i in range(HT):
                    for ch in range(2):
                        nc.tensor.matmul(
                            ps[:, ch * 512 : (ch + 1) * 512],
                            lhsT=h_sb[i][:, t * P : (t + 1) * P],
                            rhs=wfc2T[i][:, ch * 512 : (ch + 1) * 512],
                            start=(i == 0),
                            stop=(i == HT - 1),
                        )
                o = out_pool.tile([P, C], F32, tag="o", name="o")
                nc.vector.tensor_add(o[:], ps[:], x_sb[t][:])
                nc.vector.tensor_add(o[:], o[:], bfc2B[:])
                nc.sync.dma_start(out_d[t * P : (t + 1) * P, :], o[:])

        es_mlp.close()

    nc.finalize()
    return nc


_NC_CACHE = None


def _get_graph():
    global _NC_CACHE
    if _NC_CACHE is None:
        _NC_CACHE = build_graph()
    return _NC_CACHE


def make_in_maps(inputs):
    bf = ml_dtypes.bfloat16
    f32 = np.float32
    x = np.ascontiguousarray(np.asarray(inputs["x"], f32))
    wqkvT = np.ascontiguousarray(np.asarray(inputs["w_qkv"], f32).T).astype(bf)
    wprojT = np.ascontiguousarray(np.asarray(inputs["w_proj"], f32).T).astype(bf)
    wfc1T = np.ascontiguousarray(np.asarray(inputs["w_fc1"], f32).T).astype(bf)
    wfc2T = np.ascontiguousarray(np.asarray(inputs["w_fc2"], f32).T).astype(bf)
    ln1g = np.ascontiguousarray(np.asarray(inputs["ln1_g"], f32).reshape(CT, P).T)
    ln1b = np.ascontiguousarray(np.asarray(inputs["ln1_b"], f32).reshape(CT, P).T)
    ln2g = np.ascontiguousarray(np.asarray(inputs["ln2_g"], f32).reshape(CT, P).T)
    ln2b = np.ascontiguousarray(np.asarray(inputs["ln2_b"], f32).reshape(CT, P).T)
    bfc1 = np.ascontiguousarray(np.asarray(inputs["b_fc1"], f32).reshape(HT, P).T)
    bprojB = np.ascontiguousarray(
        np.broadcast_to(np.asarray(inputs["b_proj"], f32), (P, C))
    )
    bfc2B = np.ascontiguousarray(
        np.broadcast_to(np.asarray(inputs["b_fc2"], f32), (P, C))
    )
    sel2 = np.zeros((2, P), f32)
    sel2[0, :D] = 1.0
    sel2[1, D:] = 1.0

    shared = dict(
        wqkvT=wqkvT,
        wprojT=wprojT,
        wfc1T=wfc1T,
        wfc2T=wfc2T,
        ln1g=ln1g,
        ln1b=ln1b,
        ln2g=ln2g,
        ln2b=ln2b,
        bfc1=bfc1,
        bprojB=bprojB,
        bfc2B=bfc2B,
        sel2=sel2,
    )
    return [dict(shared, x=np.ascontiguousarray(x[i])) for i in range(B)]


def kernel(**inputs):
    nc = _get_graph()
    in_maps = make_in_maps(inputs)
    res = run_bass_kernel_spmd(nc, in_maps, core_ids=list(range(B)))
    out = np.stack([np.asarray(res.results[i]["out"]) for i in range(B)])
    attn = np.stack([np.asarray(res.results[i]["attn"]) for i in range(B)])
    return out.astype(np.float32), attn.astype(np.float32)
